# revision 35
# baseline (speedup 1.0000x reference)
"""MoE block kernel for Trainium2 (8 NeuronCores, data-parallel over tokens).

Reference semantics (faithful to the source module's quirk): the 4 expert ids
are taken from token (0,0)'s router logits and applied to the WHOLE batch;
per-token softmax weights over each token's own top-4 logit values still apply.

Strategy (v2 — hybrid fp8 DoubleRow up-projection):
  host: compute the 4 expert ids (tiny dot product), gather + pre-transpose
        those experts' weights, shard tokens 8 ways.
        Up-proj weights are split along the contraction (H): the first
        2*NDR h-tiles are quantized to fp8-e4m3 (scaled by WS=512 to avoid
        subnormals), the remaining h-tiles stay f32r but are ALSO scaled by
        WS (exact, power of two) so both halves share one PSUM accumulation
        group.  The 1/WS unscale is folded into the SwiGLU clamp ops
        (b_up/b_down are zero for this problem — asserted).
  device (per core, 1024 tokens):
    - RMSNorm in token-major, PE-transpose to feature-major xnT [h, t]
      (f32r copy for router + f32r matmuls, fp8 copy for DoubleRow matmuls)
    - router logits via PE in f32r (exact), per-token top-4 + softmax
    - per expert: up-proj = NDR DoubleRow fp8 matmuls (2 h-tiles each at
      ~2x rate) + (HT-2*NDR) f32r matmuls into one PSUM group -> SwiGLU ->
      actT (f32r); down-proj f32r with act stationary -> token-major out
      weighted by per-token probs, accumulated into acc (seeded with x).
    - final out DMA per token-subtile overlapped with the last expert.
"""

import numpy as np
import ml_dtypes

import concourse.bass as bass
import concourse.bacc as bacc
import concourse.mybir as mybir
import concourse.tile as tile
from concourse.bass_utils import run_bass_kernel_spmd
from concourse.masks import make_identity

F32 = mybir.dt.float32
F32R = mybir.dt.float32r
F8E4 = mybir.dt.float8e4
BF16 = mybir.dt.bfloat16
AX = mybir.AxisListType
ALU = mybir.AluOpType
ACTF = mybir.ActivationFunctionType
DR = mybir.MatmulPerfMode.DoubleRow
NP_F8 = ml_dtypes.float8_e4m3  # TRN FP8_EXP4: max normal 240, inf beyond

# problem shapes (hardcoded per contract)
B, S, H, I2, E = 4, 2048, 1536, 6144, 16
I = I2 // 2          # 3072
NE = 4               # experts applied (top-4 of token (0,0))
N_CORES = 8
T_FULL = B * S       # 8192 tokens
T = T_FULL // N_CORES  # 1024 tokens per core

ALPHA = 1.702
LIMIT = 7.0
EPS = 1e-6

HT = H // 128        # 12 h-tiles
# Asymmetric fp8 fraction: the l (linear) half of SwiGLU tolerates fp8 noise
# much better than the g (gated) half — sim: g=4,l=6 pairs → 1.80e-2 < 2e-2.
NDR_G = 4            # h-tile PAIRS in fp8 DoubleRow for the g columns
NDR_L = 6            # h-tile PAIRS in fp8 DoubleRow for the l columns (all)
NF32_G = HT - 2 * NDR_G   # trailing f32r h-tiles for g
NF32_L = HT - 2 * NDR_L   # 0
WS = 512.0           # weight pre-scale before fp8 quant (power of 2)
INV_WS = 1.0 / WS
F8MAX = 240.0        # TRN FP8_EXP4 max normal


def _chunks(n, size):
    out = []
    i = 0
    while i < n:
        out.append(list(range(i, min(i + size, n))))
        i += size
    return out


def build_moe_nc(t_tokens=T, n_experts=NE):
    """Build the per-core Bass program. Same program runs SPMD on all cores."""
    nc = bacc.Bacc(trn_type="TRN2")

    NTT = t_tokens // 128    # token subtiles
    TCH = min(512, t_tokens)  # token chunk for up-proj moving dim
    NTC = t_tokens // TCH
    HCH = 512                # h chunk for down-proj moving dim
    NHC = H // HCH           # 3
    NPAIR = I2 // 256        # 24 (g block j pairs with l block j+24)
    PAIR_GROUPS = _chunks(NPAIR, 6)    # act groups resident in SBUF
    TS_GROUPS = _chunks(NTT, 4)        # down-proj psum groups

    # x_sh is f32 data; declared f32r so x_t can feed the fused
    # transpose+rstd f32r matmul (non-matmul users bitcast back to f32)
    x_sh = nc.dram_tensor("x_sh", [t_tokens, H], F32R, kind="ExternalInput").ap()
    wu8g = nc.dram_tensor(
        "wu8g", [n_experts, NPAIR, 128, NDR_G, 2, 128], F8E4, kind="ExternalInput"
    ).ap()
    wufg = nc.dram_tensor(
        "wufg", [n_experts, NPAIR, 128, NF32_G, 128], F32R, kind="ExternalInput"
    ).ap()
    wu8l = nc.dram_tensor(
        "wu8l", [n_experts, NPAIR, 128, NDR_L, 2, 128], F8E4, kind="ExternalInput"
    ).ap()
    wdT = nc.dram_tensor("wdT", [n_experts, I, H], F32R, kind="ExternalInput").ap()
    gwD = nc.dram_tensor("gwD", [128, H // 128, E], F32R, kind="ExternalInput").ap()
    out_sh = nc.dram_tensor("out_sh", [t_tokens, H], F32, kind="ExternalOutput").ap()

    with tile.TileContext(nc) as tc:
        with (
            tc.tile_pool(name="const", bufs=1) as const,
            tc.tile_pool(name="xnT_p", bufs=1) as xnT_p,
            tc.tile_pool(name="xnT8_p", bufs=1) as xnT8_p,
            tc.tile_pool(name="actT_p", bufs=1) as actT_p,
            tc.tile_pool(name="acc_p", bufs=1) as acc_p,
            tc.tile_pool(name="xio", bufs=4) as xio,
            tc.tile_pool(name="wu8g_p", bufs=3) as wu8g_p,
            tc.tile_pool(name="wufg_p", bufs=3) as wufg_p,
            tc.tile_pool(name="wu8l_p", bufs=3) as wu8l_p,
            tc.tile_pool(name="wd_p", bufs=12) as wd_p,
            tc.tile_pool(name="tmp", bufs=1) as tmp,
            tc.tile_pool(name="rtr", bufs=2) as rtr,
        ):
            # constants
            identity = const.tile([128, 128], F32)
            make_identity(nc, identity)
            eps_t = const.tile([128, 1], F32)
            nc.vector.memset(eps_t, EPS)
            gw_sb = const.tile([128, HT, E], F32R)
            nc.sync.dma_start(out=gw_sb, in_=gwD)
            # pre-warm ACT LUTs so table loads hide under the first DMAs
            warm = const.tile([128, 1], F32)
            for fn in (ACTF.Square, ACTF.Sqrt, ACTF.Sigmoid, ACTF.Exp):
                nc.scalar.activation(out=warm, in_=eps_t, func=fn)

            xnT = xnT_p.tile([128, HT, t_tokens], F32R)
            xnT8 = xnT8_p.tile([128, HT, t_tokens], F8E4)
            actT = actT_p.tile([128, len(PAIR_GROUPS[0]), t_tokens], F32R)
            acc = acc_p.tile([128, NTT, H], F32)
            probs = rtr.tile([128, NTT, n_experts], F32, tag="probs", bufs=1)

            def prologue_norm(ts):
                """x DMA, compute rstd; build diag(rstd) for the fused
                transpose+scale matmul. x_t stays un-normalized."""
                x_t = xio.tile([128, H], F32R, tag="x_t", name="x_t")
                nc.sync.dma_start(out=x_t, in_=x_sh[ts * 128:(ts + 1) * 128, :])
                ss = rtr.tile([128, 1], F32, tag="ss", name="ss")
                nc.scalar.activation(
                    out=acc[:, ts, :], in_=x_t.bitcast(F32), func=ACTF.Square,
                    accum_out=ss,
                )
                rt = rtr.tile([128, 1], F32, tag="rt", name="rt")
                nc.scalar.activation(
                    out=rt, in_=ss, func=ACTF.Sqrt, bias=eps_t, scale=1.0 / H
                )
                nc.vector.reciprocal(rt, rt)
                diag = rtr.tile([128, 128], F32R, tag="diag", name="diag", bufs=4)
                nc.vector.tensor_scalar_mul(diag, identity, rt)
                return x_t, diag

            def prologue_tr(ts, xd, ps_pool):
                # xnT[h, t] = sum_tin x[tin, h] * rstd[tin] * I[tin, t]
                # — PE matmul fuses the transpose with the rstd scaling.
                x_t, diag = xd
                for ht in range(HT):
                    tr_ps = ps_pool.tile([128, 128], F32, tag="up", name="tr_ps")
                    nc.tensor.matmul(
                        tr_ps,
                        lhsT=x_t[:, ht * 128:(ht + 1) * 128],
                        rhs=diag,
                        start=True, stop=True,
                    )
                    nc.vector.tensor_copy(
                        xnT[:, ht, ts * 128:(ts + 1) * 128], tr_ps
                    )
                    nc.vector.tensor_copy(
                        xnT8[:, ht, ts * 128:(ts + 1) * 128], tr_ps
                    )

            def router(ts, ps_pool):
                lg_ps = ps_pool.tile([128, E], F32, tag="dn", name="lg_ps")
                for ht in range(HT):
                    nc.tensor.matmul(
                        lg_ps,
                        lhsT=xnT[:, ht, ts * 128:(ts + 1) * 128],
                        rhs=gw_sb[:, ht, :],
                        start=(ht == 0),
                        stop=(ht == HT - 1),
                    )
                lg = rtr.tile([128, E], F32, tag="lg_sb", name="lg")
                nc.vector.tensor_copy(lg, lg_ps)
                vt = rtr.tile([128, n_experts], F32, tag="vt", name="vt")
                nc.vector.reduce_max(out=vt[:, 0:1], in_=lg, axis=AX.X)
                msk = rtr.tile([128, E], F32, tag="msk", name="msk")
                for k in range(1, n_experts):
                    nc.vector.tensor_scalar(
                        out=msk, in0=lg, scalar1=vt[:, k - 1:k], scalar2=1e30,
                        op0=ALU.is_equal, op1=ALU.mult,
                    )
                    nc.vector.tensor_sub(lg, lg, msk)
                    nc.vector.reduce_max(out=vt[:, k:k + 1], in_=lg, axis=AX.X)
                neg1 = rtr.tile([128, 1], F32, tag="neg1", name="neg1")
                nc.vector.tensor_scalar_mul(neg1, vt[:, 0:1], -1.0)
                ev = rtr.tile([128, n_experts], F32, tag="ev", name="ev")
                nc.scalar.activation(out=ev, in_=vt, func=ACTF.Exp, bias=neg1)
                sm = rtr.tile([128, 1], F32, tag="sm", name="sm")
                nc.vector.reduce_sum(out=sm, in_=ev, axis=AX.X)
                nc.vector.reciprocal(sm, sm)
                nc.vector.tensor_scalar_mul(probs[:, ts, :], ev, sm)

            def load_wu(e, j):
                wu8g_t = wu8g_p.tile([128, NDR_G, 2, 128], F8E4, tag="wu8g",
                                     name="wu8g_t")
                nc.sync.dma_start(out=wu8g_t, in_=wu8g[e, j])
                wufg_t = wufg_p.tile([128, NF32_G, 128], F32R, tag="wufg",
                                     name="wufg_t")
                nc.sync.dma_start(out=wufg_t, in_=wufg[e, j])
                wu8l_t = wu8l_p.tile([128, NDR_L, 2, 128], F8E4, tag="wu8l",
                                     name="wu8l_t")
                nc.sync.dma_start(out=wu8l_t, in_=wu8l[e, j])
                return wu8g_t, wufg_t, wu8l_t

            def up_pair_tc(e, j, jj, wu_t, tci, up_ps):
                wu8g_t, wufg_t, wu8l_t = wu_t
                tsl = slice(tci * TCH, (tci + 1) * TCH)
                ps_g = up_ps.tile([128, TCH], F32, tag="up", name="ps_g")
                ps_l = up_ps.tile([128, TCH], F32, tag="up", name="ps_l")
                for u in range(NDR_G):
                    nc.tensor.matmul(
                        ps_g, lhsT=wu8g_t[:, u],
                        rhs=xnT8[:, 2 * u:2 * u + 2, tsl],
                        start=(u == 0), stop=False, perf_mode=DR,
                    )
                for t in range(NF32_G):
                    nc.tensor.matmul(
                        ps_g, lhsT=wufg_t[:, t],
                        rhs=xnT[:, 2 * NDR_G + t, tsl],
                        start=False, stop=(t == NF32_G - 1),
                    )
                for u in range(NDR_L):
                    nc.tensor.matmul(
                        ps_l, lhsT=wu8l_t[:, u],
                        rhs=xnT8[:, 2 * u:2 * u + 2, tsl],
                        start=(u == 0), stop=(u == NDR_L - 1), perf_mode=DR,
                    )
                up_swiglu(jj, tci, ps_g, ps_l)

            def up_swiglu(jj, tci, ps_g, ps_l):
                tsl = slice(tci * TCH, (tci + 1) * TCH)
                tg = tmp.tile([128, TCH], F32, tag="tg", name="tg")
                nc.vector.tensor_scalar(
                    out=tg, in0=ps_g, scalar1=INV_WS, scalar2=LIMIT,
                    op0=ALU.mult, op1=ALU.min,
                )
                tsg = tmp.tile([128, TCH], F32, tag="tsg", name="tsg")
                nc.scalar.activation(out=tsg, in_=tg, func=ACTF.Sigmoid, scale=ALPHA)
                nc.vector.tensor_mul(tg, tg, tsg)
                tl = tmp.tile([128, TCH], F32, tag="tl", name="tl")
                nc.vector.tensor_scalar(
                    out=tl, in0=ps_l, scalar1=INV_WS, scalar2=LIMIT,
                    op0=ALU.mult, op1=ALU.min,
                )
                nc.vector.tensor_scalar(
                    out=tl, in0=tl, scalar1=-LIMIT, scalar2=1.0,
                    op0=ALU.max, op1=ALU.add,
                )
                nc.vector.tensor_mul(actT[:, jj, tsl], tg, tl)

            def load_wd(e, grp, hc):
                wd_ts = []
                for it in range(len(grp)):
                    i_glob = grp[it]
                    wd_t = wd_p.tile([128, HCH], F32R, tag="wd",
                                     name="wd_t")
                    nc.sync.dma_start(
                        out=wd_t,
                        in_=wdT[
                            e,
                            i_glob * 128:(i_glob + 1) * 128,
                            hc * HCH:(hc + 1) * HCH,
                        ],
                    )
                    wd_ts.append(wd_t)
                return wd_ts

            def down_group(e, grp, dn_ps, emit_out=False, wd_pre=None):
                # single pass over w_down: each h-chunk's tiles are held in
                # SBUF across all token-subtile groups
                for hc in range(NHC):
                    if wd_pre is not None and wd_pre.get(hc) is not None:
                        wd_ts = wd_pre[hc]
                    else:
                        wd_ts = load_wd(e, grp, hc)
                    hsl = slice(hc * HCH, (hc + 1) * HCH)
                    # finer chunks on the very last pass so the out DMAs
                    # overlap the remaining matmuls instead of trailing
                    ts_groups = (
                        _chunks(NTT, 1)
                        if emit_out and hc == NHC - 1 else TS_GROUPS
                    )
                    for tsg in ts_groups:
                        ps_os = [
                            dn_ps.tile([128, HCH], F32, tag="dn",
                                       name=f"ps_o{k}")
                            for k in range(len(tsg))
                        ]
                        for it in range(len(grp)):
                            for k, ts in enumerate(tsg):
                                nc.tensor.matmul(
                                    ps_os[k],
                                    lhsT=actT[:, it, ts * 128:(ts + 1) * 128],
                                    rhs=wd_ts[it],
                                    start=(it == 0),
                                    stop=(it == len(grp) - 1),
                                )
                        for k, ts in enumerate(tsg):
                            nc.vector.scalar_tensor_tensor(
                                out=acc[:, ts, hsl],
                                in0=ps_os[k],
                                scalar=probs[:, ts, e:e + 1],
                                in1=acc[:, ts, hsl],
                                op0=ALU.mult,
                                op1=ALU.add,
                            )
                            if emit_out:
                                # acc[:, ts, hsl] is final — stream it out now
                                nc.sync.dma_start(
                                    out=out_sh[ts * 128:(ts + 1) * 128, hsl],
                                    in_=acc[:, ts, hsl],
                                )

            # ---------------- prologue + pipelined expert 0 group 0 --------
            with (
                tc.tile_pool(name="up_ps", bufs=4, space="PSUM") as up_ps,
                tc.tile_pool(name="dn_ps", bufs=4, space="PSUM") as dn_ps,
            ):
                grp0 = PAIR_GROUPS[0]
                if NTT == 8 and NTC == 2:
                    # x DMAs first (the norm/diag chain gates the first
                    # transposes), then expert-0's first weight tiles
                    xds = [prologue_norm(ts) for ts in range(4)]
                    wu_h = {}
                    for jj, j in enumerate(grp0[:3]):
                        wu_h[jj] = load_wu(0, j)
                    for ts in range(4):
                        prologue_tr(ts, xds[ts], up_ps)
                    up_pair_tc(0, grp0[0], 0, wu_h[0], 0, up_ps)
                    x_ts = {}
                    x_ts[4] = prologue_norm(4)
                    x_ts[5] = prologue_norm(5)
                    for jj, j in enumerate(grp0[1:3], start=1):
                        up_pair_tc(0, j, jj, wu_h[jj], 0, up_ps)
                    prologue_tr(4, x_ts[4], up_ps)
                    prologue_tr(5, x_ts[5], up_ps)
                    # weight loads go on the queue ahead of the x6/x7 burst
                    for jj, j in enumerate(grp0[3:], start=3):
                        wu_h[jj] = load_wu(0, j)
                    x_ts[6] = prologue_norm(6)
                    x_ts[7] = prologue_norm(7)
                    for jj, j in enumerate(grp0[3:], start=3):
                        up_pair_tc(0, j, jj, wu_h[jj], 0, up_ps)
                    prologue_tr(6, x_ts[6], up_ps)
                    prologue_tr(7, x_ts[7], up_ps)
                    # seed acc = x now that the startup DMA burst is over
                    for ts in range(NTT):
                        nc.sync.dma_start(
                            out=acc[:, ts, :],
                            in_=x_sh[ts * 128:(ts + 1) * 128, :].bitcast(F32),
                        )
                    for ts in range(4):
                        router(ts, dn_ps)
                    # tc1: reuse the last 3 wu tiles still resident (reverse
                    # order), reload the rest
                    n0 = len(grp0)
                    for jj in range(n0 - 1, n0 - 4, -1):
                        up_pair_tc(0, grp0[jj], jj, wu_h[jj], 1, up_ps)
                    for ts in range(4, NTT):
                        router(ts, dn_ps)
                    for jj in range(n0 - 4, -1, -1):
                        wu_t = load_wu(0, grp0[jj])
                        up_pair_tc(0, grp0[jj], jj, wu_t, 1, up_ps)
                else:
                    for ts in range(NTT):
                        xd = prologue_norm(ts)
                        prologue_tr(ts, xd, up_ps)
                    for ts in range(NTT):
                        nc.sync.dma_start(
                            out=acc[:, ts, :],
                            in_=x_sh[ts * 128:(ts + 1) * 128, :].bitcast(F32),
                        )
                    for jj, j in enumerate(grp0):
                        wu_t = load_wu(0, j)
                        for tci in range(NTC):
                            up_pair_tc(0, j, jj, wu_t, tci, up_ps)
                    for ts in range(NTT):
                        router(ts, dn_ps)
                down_group(0, grp0, dn_ps)

                for e in range(n_experts):
                    for gi, grp in enumerate(PAIR_GROUPS):
                        if e == 0 and gi == 0:
                            continue
                        last = (e == n_experts - 1
                                and gi == len(PAIR_GROUPS) - 1)
                        wd_pre = {}
                        for jj, j in enumerate(grp):
                            wu_t = load_wu(e, j)
                            for tci in range(NTC):
                                up_pair_tc(e, j, jj, wu_t, tci, up_ps)
                            if jj == 3:
                                wd_pre[0] = load_wd(e, grp, 0)
                            elif last and jj == 4:
                                wd_pre[1] = load_wd(e, grp, 1)
                            elif last and jj == 5:
                                wd_pre[2] = load_wd(e, grp, 2)
                        down_group(e, grp, dn_ps, emit_out=last,
                                   wd_pre=wd_pre)

    nc.compile()
    return nc


_NC_CACHE = {}


def _get_nc(t_tokens=T, n_experts=NE):
    key = (t_tokens, n_experts)
    if key not in _NC_CACHE:
        _NC_CACHE[key] = build_moe_nc(t_tokens, n_experts)
    return _NC_CACHE[key]


def _prepare_host(x, norm_scale, gate_w, w_up, b_up, w_down, b_down):
    """Routing + weight gather/quantization on host. Returns per-core in_maps."""
    x = np.asarray(x, dtype=np.float32)
    norm_scale = np.asarray(norm_scale, dtype=np.float32)
    gate_w = np.asarray(gate_w, dtype=np.float32)

    assert not np.any(np.asarray(b_up)), "kernel assumes b_up == 0"
    assert not np.any(np.asarray(b_down)), "kernel assumes b_down == 0"

    x00 = x.reshape(-1, H)[0].astype(np.float64)
    rstd = 1.0 / np.sqrt(np.mean(x00 * x00) + EPS)
    xn00 = x00 * rstd * norm_scale.astype(np.float64)
    logits00 = gate_w.astype(np.float64) @ xn00
    eids = np.argsort(-logits00, kind="stable")[:NE] % E

    wu = np.asarray(w_up, dtype=np.float32)[eids]     # [NE, I2, H]
    wd = np.asarray(w_down, dtype=np.float32)[eids]   # [NE, H, I]
    gw = gate_w
    if not np.all(norm_scale == 1.0):
        # fold the RMSNorm scale into every weight that contracts over H
        wu = wu * norm_scale[None, None, :]
        gw = gate_w * norm_scale[None, :]

    NPAIR = I2 // 256
    wu_s = wu * np.float32(WS)
    wg, wl = wu_s[:, :I], wu_s[:, I:]          # SwiGLU gate / linear halves
    # [e, i2h, h] -> [e, j, m, u, q, p]; device wants [e, j, p, u(pairs), q, m]
    wg_r = wg.reshape(NE, NPAIR, 128, HT // 2, 2, 128)
    wl_r = wl.reshape(NE, NPAIR, 128, HT // 2, 2, 128)
    wu8gD = np.ascontiguousarray(
        wg_r[:, :, :, :NDR_G].transpose(0, 1, 5, 3, 4, 2)
    )
    wu8gD = np.clip(wu8gD, -F8MAX, F8MAX).astype(NP_F8)
    wu8lD = np.ascontiguousarray(
        wl_r[:, :, :, :NDR_L].transpose(0, 1, 5, 3, 4, 2)
    )
    wu8lD = np.clip(wu8lD, -F8MAX, F8MAX).astype(NP_F8)
    # f32r tail of the g contraction (also scaled by WS — exact)
    wufgD = np.ascontiguousarray(
        wg[:, :, 2 * NDR_G * 128:]
        .reshape(NE, NPAIR, 128, NF32_G, 128)
        .transpose(0, 1, 4, 3, 2)
    )

    wdT = np.ascontiguousarray(wd.transpose(0, 2, 1))  # [NE, I, H]
    gwT = np.ascontiguousarray(gw.T)                   # [H, E]
    gwD = np.ascontiguousarray(
        gwT.reshape(H // 128, 128, E).transpose(1, 0, 2)
    )                                                  # [128, HT, E]

    x_flat = np.ascontiguousarray(x.reshape(T_FULL, H))
    in_maps = []
    for c in range(N_CORES):
        in_maps.append(
            {
                "x_sh": x_flat[c * T:(c + 1) * T],
                "wu8g": wu8gD,
                "wufg": wufgD,
                "wu8l": wu8lD,
                "wdT": wdT,
                "gwD": gwD,
            }
        )
    return in_maps, x.shape


def run_moe(inputs, trace=False, **run_kwargs):
    in_maps, x_shape = _prepare_host(**inputs)
    nc = _get_nc()
    br = run_bass_kernel_spmd(
        nc, in_maps, core_ids=list(range(N_CORES)), trace=trace, **run_kwargs
    )
    out = np.concatenate([r["out_sh"] for r in br.results], axis=0)
    return out.reshape(x_shape), br


def kernel(**inputs) -> np.ndarray:
    out, _ = run_moe(inputs, trace=False)
    return out


# revision 37
# speedup vs baseline: 1.0073x; 1.0073x over previous
"""MoE block kernel for Trainium2 (8 NeuronCores, data-parallel over tokens).

Reference semantics (faithful to the source module's quirk): the 4 expert ids
are taken from token (0,0)'s router logits and applied to the WHOLE batch;
per-token softmax weights over each token's own top-4 logit values still apply.

Strategy (v2 — hybrid fp8 DoubleRow up-projection):
  host: compute the 4 expert ids (tiny dot product), gather + pre-transpose
        those experts' weights, shard tokens 8 ways.
        Up-proj weights are split along the contraction (H): the first
        2*NDR h-tiles are quantized to fp8-e4m3 (scaled by WS=512 to avoid
        subnormals), the remaining h-tiles stay f32r but are ALSO scaled by
        WS (exact, power of two) so both halves share one PSUM accumulation
        group.  The 1/WS unscale is folded into the SwiGLU clamp ops
        (b_up/b_down are zero for this problem — asserted).
  device (per core, 1024 tokens):
    - RMSNorm in token-major, PE-transpose to feature-major xnT [h, t]
      (f32r copy for router + f32r matmuls, fp8 copy for DoubleRow matmuls)
    - router logits via PE in f32r (exact), per-token top-4 + softmax
    - per expert: up-proj = NDR DoubleRow fp8 matmuls (2 h-tiles each at
      ~2x rate) + (HT-2*NDR) f32r matmuls into one PSUM group -> SwiGLU ->
      actT (f32r); down-proj f32r with act stationary -> token-major out
      weighted by per-token probs, accumulated into acc (seeded with x).
    - final out DMA per token-subtile overlapped with the last expert.
"""

import numpy as np
import ml_dtypes

import concourse.bass as bass
import concourse.bacc as bacc
import concourse.mybir as mybir
import concourse.tile as tile
from concourse.bass_utils import run_bass_kernel_spmd
from concourse.masks import make_identity

F32 = mybir.dt.float32
F32R = mybir.dt.float32r
F8E4 = mybir.dt.float8e4
BF16 = mybir.dt.bfloat16
AX = mybir.AxisListType
ALU = mybir.AluOpType
ACTF = mybir.ActivationFunctionType
DR = mybir.MatmulPerfMode.DoubleRow
NP_F8 = ml_dtypes.float8_e4m3  # TRN FP8_EXP4: max normal 240, inf beyond

# problem shapes (hardcoded per contract)
B, S, H, I2, E = 4, 2048, 1536, 6144, 16
I = I2 // 2          # 3072
NE = 4               # experts applied (top-4 of token (0,0))
N_CORES = 8
T_FULL = B * S       # 8192 tokens
T = T_FULL // N_CORES  # 1024 tokens per core

ALPHA = 1.702
LIMIT = 7.0
EPS = 1e-6

HT = H // 128        # 12 h-tiles
# Asymmetric fp8 fraction: the l (linear) half of SwiGLU tolerates fp8 noise
# much better than the g (gated) half — sim: g=4,l=6 pairs → 1.80e-2 < 2e-2.
NDR_G = 4            # h-tile PAIRS in fp8 DoubleRow for the g columns
NDR_L = 6            # h-tile PAIRS in fp8 DoubleRow for the l columns (all)
NF32_G = HT - 2 * NDR_G   # trailing f32r h-tiles for g
NF32_L = HT - 2 * NDR_L   # 0
WS = 512.0           # weight pre-scale before fp8 quant (power of 2)
INV_WS = 1.0 / WS
F8MAX = 240.0        # TRN FP8_EXP4 max normal


def _chunks(n, size):
    out = []
    i = 0
    while i < n:
        out.append(list(range(i, min(i + size, n))))
        i += size
    return out


def build_moe_nc(t_tokens=T, n_experts=NE):
    """Build the per-core Bass program. Same program runs SPMD on all cores."""
    nc = bacc.Bacc(trn_type="TRN2")

    NTT = t_tokens // 128    # token subtiles
    TCH = min(512, t_tokens)  # token chunk for up-proj moving dim
    NTC = t_tokens // TCH
    HCH = 512                # h chunk for down-proj moving dim
    NHC = H // HCH           # 3
    NPAIR = I2 // 256        # 24 (g block j pairs with l block j+24)
    PAIR_GROUPS = _chunks(NPAIR, 6)    # act groups resident in SBUF
    TS_GROUPS = _chunks(NTT, 4)        # down-proj psum groups

    # x_sh is f32 data; declared f32r so x_t can feed the fused
    # transpose+rstd f32r matmul (non-matmul users bitcast back to f32)
    x_sh = nc.dram_tensor("x_sh", [t_tokens, H], F32R, kind="ExternalInput").ap()
    wu8g = nc.dram_tensor(
        "wu8g", [n_experts, NPAIR, 128, NDR_G, 2, 128], F8E4, kind="ExternalInput"
    ).ap()
    wufg = nc.dram_tensor(
        "wufg", [n_experts, NPAIR, 128, NF32_G, 128], F32R, kind="ExternalInput"
    ).ap()
    wu8l = nc.dram_tensor(
        "wu8l", [n_experts, NPAIR, 128, NDR_L, 2, 128], F8E4, kind="ExternalInput"
    ).ap()
    wdT = nc.dram_tensor("wdT", [n_experts, I, H], F32R, kind="ExternalInput").ap()
    gwD = nc.dram_tensor("gwD", [128, H // 128, E], F32R, kind="ExternalInput").ap()
    out_sh = nc.dram_tensor("out_sh", [t_tokens, H], F32, kind="ExternalOutput").ap()

    with tile.TileContext(nc) as tc:
        with (
            tc.tile_pool(name="const", bufs=1) as const,
            tc.tile_pool(name="xnT_p", bufs=1) as xnT_p,
            tc.tile_pool(name="xnT8_p", bufs=1) as xnT8_p,
            tc.tile_pool(name="actT_p", bufs=1) as actT_p,
            tc.tile_pool(name="acc_p", bufs=1) as acc_p,
            tc.tile_pool(name="xio", bufs=4) as xio,
            tc.tile_pool(name="wu8g_p", bufs=3) as wu8g_p,
            tc.tile_pool(name="wufg_p", bufs=3) as wufg_p,
            tc.tile_pool(name="wu8l_p", bufs=3) as wu8l_p,
            tc.tile_pool(name="wd_p", bufs=12) as wd_p,
            tc.tile_pool(name="tmp", bufs=1) as tmp,
            tc.tile_pool(name="rtr", bufs=2) as rtr,
        ):
            # constants
            identity = const.tile([128, 128], F32)
            make_identity(nc, identity)
            eps_t = const.tile([128, 1], F32)
            nc.vector.memset(eps_t, EPS)
            gw_sb = const.tile([128, HT, E], F32R)
            nc.sync.dma_start(out=gw_sb, in_=gwD)
            # pre-warm ACT LUTs so table loads hide under the first DMAs
            warm = const.tile([128, 1], F32)
            for fn in (ACTF.Square, ACTF.Sqrt, ACTF.Sigmoid, ACTF.Exp):
                nc.scalar.activation(out=warm, in_=eps_t, func=fn)

            xnT = xnT_p.tile([128, HT, t_tokens], F32R)
            xnT8 = xnT8_p.tile([128, HT, t_tokens], F8E4)
            actT = actT_p.tile([128, len(PAIR_GROUPS[0]), t_tokens], F32R)
            acc = acc_p.tile([128, NTT, H], F32)
            probs = rtr.tile([128, NTT, n_experts], F32, tag="probs", bufs=1)

            def prologue_norm(ts):
                """x DMA, compute rstd; build diag(rstd) for the fused
                transpose+scale matmul. x_t stays un-normalized."""
                x_t = xio.tile([128, H], F32R, tag="x_t", name="x_t")
                nc.sync.dma_start(out=x_t, in_=x_sh[ts * 128:(ts + 1) * 128, :])
                ss = rtr.tile([128, 1], F32, tag="ss", name="ss")
                nc.scalar.activation(
                    out=acc[:, ts, :], in_=x_t.bitcast(F32), func=ACTF.Square,
                    accum_out=ss,
                )
                rt = rtr.tile([128, 1], F32, tag="rt", name="rt")
                nc.scalar.activation(
                    out=rt, in_=ss, func=ACTF.Sqrt, bias=eps_t, scale=1.0 / H
                )
                nc.vector.reciprocal(rt, rt)
                diag = rtr.tile([128, 128], F32R, tag="diag", name="diag", bufs=4)
                nc.vector.tensor_scalar_mul(diag, identity, rt)
                return x_t, diag

            def prologue_tr(ts, xd, ps_pool):
                # xnT[h, t] = sum_tin x[tin, h] * rstd[tin] * I[tin, t]
                # — PE matmul fuses the transpose with the rstd scaling.
                x_t, diag = xd
                for ht in range(HT):
                    tr_ps = ps_pool.tile([128, 128], F32, tag="up", name="tr_ps")
                    nc.tensor.matmul(
                        tr_ps,
                        lhsT=x_t[:, ht * 128:(ht + 1) * 128],
                        rhs=diag,
                        start=True, stop=True,
                    )
                    nc.vector.tensor_copy(
                        xnT[:, ht, ts * 128:(ts + 1) * 128], tr_ps
                    )
                    nc.vector.tensor_copy(
                        xnT8[:, ht, ts * 128:(ts + 1) * 128], tr_ps
                    )

            def router(ts, ps_pool):
                lg_ps = ps_pool.tile([128, E], F32, tag="dn", name="lg_ps")
                for ht in range(HT):
                    nc.tensor.matmul(
                        lg_ps,
                        lhsT=xnT[:, ht, ts * 128:(ts + 1) * 128],
                        rhs=gw_sb[:, ht, :],
                        start=(ht == 0),
                        stop=(ht == HT - 1),
                    )
                lg = rtr.tile([128, E], F32, tag="lg_sb", name="lg")
                nc.vector.tensor_copy(lg, lg_ps)
                vt = rtr.tile([128, n_experts], F32, tag="vt", name="vt")
                nc.vector.reduce_max(out=vt[:, 0:1], in_=lg, axis=AX.X)
                msk = rtr.tile([128, E], F32, tag="msk", name="msk")
                for k in range(1, n_experts):
                    nc.vector.tensor_scalar(
                        out=msk, in0=lg, scalar1=vt[:, k - 1:k], scalar2=1e30,
                        op0=ALU.is_equal, op1=ALU.mult,
                    )
                    nc.vector.tensor_sub(lg, lg, msk)
                    nc.vector.reduce_max(out=vt[:, k:k + 1], in_=lg, axis=AX.X)
                neg1 = rtr.tile([128, 1], F32, tag="neg1", name="neg1")
                nc.vector.tensor_scalar_mul(neg1, vt[:, 0:1], -1.0)
                ev = rtr.tile([128, n_experts], F32, tag="ev", name="ev")
                nc.scalar.activation(out=ev, in_=vt, func=ACTF.Exp, bias=neg1)
                sm = rtr.tile([128, 1], F32, tag="sm", name="sm")
                nc.vector.reduce_sum(out=sm, in_=ev, axis=AX.X)
                nc.vector.reciprocal(sm, sm)
                nc.vector.tensor_scalar_mul(probs[:, ts, :], ev, sm)

            def load_wu(e, j):
                wu8g_t = wu8g_p.tile([128, NDR_G, 2, 128], F8E4, tag="wu8g",
                                     name="wu8g_t")
                nc.sync.dma_start(out=wu8g_t, in_=wu8g[e, j])
                wufg_t = wufg_p.tile([128, NF32_G, 128], F32R, tag="wufg",
                                     name="wufg_t")
                nc.sync.dma_start(out=wufg_t, in_=wufg[e, j])
                wu8l_t = wu8l_p.tile([128, NDR_L, 2, 128], F8E4, tag="wu8l",
                                     name="wu8l_t")
                nc.sync.dma_start(out=wu8l_t, in_=wu8l[e, j])
                return wu8g_t, wufg_t, wu8l_t

            def up_pair_tc(e, j, jj, wu_t, tci, up_ps):
                wu8g_t, wufg_t, wu8l_t = wu_t
                tsl = slice(tci * TCH, (tci + 1) * TCH)
                ps_g = up_ps.tile([128, TCH], F32, tag="up", name="ps_g")
                ps_l = up_ps.tile([128, TCH], F32, tag="up", name="ps_l")
                for u in range(NDR_G):
                    nc.tensor.matmul(
                        ps_g, lhsT=wu8g_t[:, u],
                        rhs=xnT8[:, 2 * u:2 * u + 2, tsl],
                        start=(u == 0), stop=False, perf_mode=DR,
                    )
                for t in range(NF32_G):
                    nc.tensor.matmul(
                        ps_g, lhsT=wufg_t[:, t],
                        rhs=xnT[:, 2 * NDR_G + t, tsl],
                        start=False, stop=(t == NF32_G - 1),
                    )
                for u in range(NDR_L):
                    nc.tensor.matmul(
                        ps_l, lhsT=wu8l_t[:, u],
                        rhs=xnT8[:, 2 * u:2 * u + 2, tsl],
                        start=(u == 0), stop=(u == NDR_L - 1), perf_mode=DR,
                    )
                up_swiglu(jj, tci, ps_g, ps_l)

            def up_swiglu(jj, tci, ps_g, ps_l):
                tsl = slice(tci * TCH, (tci + 1) * TCH)
                tg = tmp.tile([128, TCH], F32, tag="tg", name="tg")
                nc.vector.tensor_scalar(
                    out=tg, in0=ps_g, scalar1=INV_WS, scalar2=LIMIT,
                    op0=ALU.mult, op1=ALU.min,
                )
                tsg = tmp.tile([128, TCH], F32, tag="tsg", name="tsg")
                nc.scalar.activation(out=tsg, in_=tg, func=ACTF.Sigmoid, scale=ALPHA)
                nc.vector.tensor_mul(tg, tg, tsg)
                tl = tmp.tile([128, TCH], F32, tag="tl", name="tl")
                nc.vector.tensor_scalar(
                    out=tl, in0=ps_l, scalar1=INV_WS, scalar2=LIMIT,
                    op0=ALU.mult, op1=ALU.min,
                )
                nc.vector.tensor_scalar(
                    out=tl, in0=tl, scalar1=-LIMIT, scalar2=1.0,
                    op0=ALU.max, op1=ALU.add,
                )
                nc.vector.tensor_mul(actT[:, jj, tsl], tg, tl)

            def load_wd(e, grp, hc):
                wd_ts = []
                for it in range(len(grp)):
                    i_glob = grp[it]
                    wd_t = wd_p.tile([128, HCH], F32R, tag="wd",
                                     name="wd_t")
                    nc.sync.dma_start(
                        out=wd_t,
                        in_=wdT[
                            e,
                            i_glob * 128:(i_glob + 1) * 128,
                            hc * HCH:(hc + 1) * HCH,
                        ],
                    )
                    wd_ts.append(wd_t)
                return wd_ts

            def down_group(e, grp, dn_ps, emit_out=False, wd_pre=None):
                # single pass over w_down: each h-chunk's tiles are held in
                # SBUF across all token-subtile groups
                for hc in range(NHC):
                    if wd_pre is not None and wd_pre.get(hc) is not None:
                        wd_ts = wd_pre[hc]
                    else:
                        wd_ts = load_wd(e, grp, hc)
                    hsl = slice(hc * HCH, (hc + 1) * HCH)
                    # finer chunks on the very last pass so the out DMAs
                    # overlap the remaining matmuls instead of trailing
                    ts_groups = (
                        _chunks(NTT, 1)
                        if emit_out and hc == NHC - 1 else TS_GROUPS
                    )
                    for tsg in ts_groups:
                        ps_os = [
                            dn_ps.tile([128, HCH], F32, tag="dn",
                                       name=f"ps_o{k}")
                            for k in range(len(tsg))
                        ]
                        for it in range(len(grp)):
                            for k, ts in enumerate(tsg):
                                nc.tensor.matmul(
                                    ps_os[k],
                                    lhsT=actT[:, it, ts * 128:(ts + 1) * 128],
                                    rhs=wd_ts[it],
                                    start=(it == 0),
                                    stop=(it == len(grp) - 1),
                                )
                        for k, ts in enumerate(tsg):
                            nc.vector.scalar_tensor_tensor(
                                out=acc[:, ts, hsl],
                                in0=ps_os[k],
                                scalar=probs[:, ts, e:e + 1],
                                in1=acc[:, ts, hsl],
                                op0=ALU.mult,
                                op1=ALU.add,
                            )
                            if emit_out:
                                # acc[:, ts, hsl] is final — stream it out now
                                nc.sync.dma_start(
                                    out=out_sh[ts * 128:(ts + 1) * 128, hsl],
                                    in_=acc[:, ts, hsl],
                                )

            # ---------------- prologue + pipelined expert 0 group 0 --------
            with (
                tc.tile_pool(name="up_ps", bufs=4, space="PSUM") as up_ps,
                tc.tile_pool(name="dn_ps", bufs=4, space="PSUM") as dn_ps,
            ):
                grp0 = PAIR_GROUPS[0]
                wd0_pre = None
                if NTT == 8 and NTC == 2:
                    # x DMAs first (the norm/diag chain gates the first
                    # transposes), then expert-0's first weight tiles
                    xds = [prologue_norm(ts) for ts in range(4)]
                    wu_h = {}
                    for jj, j in enumerate(grp0[:3]):
                        wu_h[jj] = load_wu(0, j)
                    for ts in range(4):
                        prologue_tr(ts, xds[ts], up_ps)
                    up_pair_tc(0, grp0[0], 0, wu_h[0], 0, up_ps)
                    x_ts = {}
                    x_ts[4] = prologue_norm(4)
                    x_ts[5] = prologue_norm(5)
                    for jj, j in enumerate(grp0[1:3], start=1):
                        up_pair_tc(0, j, jj, wu_h[jj], 0, up_ps)
                    prologue_tr(4, x_ts[4], up_ps)
                    prologue_tr(5, x_ts[5], up_ps)
                    # weight loads go on the queue ahead of the x6/x7 burst
                    for jj, j in enumerate(grp0[3:], start=3):
                        wu_h[jj] = load_wu(0, j)
                    x_ts[6] = prologue_norm(6)
                    x_ts[7] = prologue_norm(7)
                    for jj, j in enumerate(grp0[3:], start=3):
                        up_pair_tc(0, j, jj, wu_h[jj], 0, up_ps)
                    prologue_tr(6, x_ts[6], up_ps)
                    prologue_tr(7, x_ts[7], up_ps)
                    # seed acc = x now that the startup DMA burst is over
                    for ts in range(NTT):
                        nc.sync.dma_start(
                            out=acc[:, ts, :],
                            in_=x_sh[ts * 128:(ts + 1) * 128, :].bitcast(F32),
                        )
                    for ts in range(4):
                        router(ts, dn_ps)
                    # tc1: reuse the last 3 wu tiles still resident (reverse
                    # order), reload the rest
                    n0 = len(grp0)
                    for jj in range(n0 - 1, n0 - 4, -1):
                        up_pair_tc(0, grp0[jj], jj, wu_h[jj], 1, up_ps)
                    for ts in range(4, NTT):
                        router(ts, dn_ps)
                    wd0_pre = None
                    for jj in range(n0 - 4, -1, -1):
                        wu_t = load_wu(0, grp0[jj])
                        up_pair_tc(0, grp0[jj], jj, wu_t, 1, up_ps)
                        if jj == 1:
                            wd0_pre = {0: load_wd(0, grp0, 0)}
                else:
                    for ts in range(NTT):
                        xd = prologue_norm(ts)
                        prologue_tr(ts, xd, up_ps)
                    for ts in range(NTT):
                        nc.sync.dma_start(
                            out=acc[:, ts, :],
                            in_=x_sh[ts * 128:(ts + 1) * 128, :].bitcast(F32),
                        )
                    for jj, j in enumerate(grp0):
                        wu_t = load_wu(0, j)
                        for tci in range(NTC):
                            up_pair_tc(0, j, jj, wu_t, tci, up_ps)
                    for ts in range(NTT):
                        router(ts, dn_ps)
                down_group(0, grp0, dn_ps, wd_pre=wd0_pre)

                for e in range(n_experts):
                    for gi, grp in enumerate(PAIR_GROUPS):
                        if e == 0 and gi == 0:
                            continue
                        last = (e == n_experts - 1
                                and gi == len(PAIR_GROUPS) - 1)
                        wd_pre = {}
                        for jj, j in enumerate(grp):
                            wu_t = load_wu(e, j)
                            for tci in range(NTC):
                                up_pair_tc(e, j, jj, wu_t, tci, up_ps)
                            if jj == 3:
                                wd_pre[0] = load_wd(e, grp, 0)
                            elif last and jj == 4:
                                wd_pre[1] = load_wd(e, grp, 1)
                            elif last and jj == 5:
                                wd_pre[2] = load_wd(e, grp, 2)
                        down_group(e, grp, dn_ps, emit_out=last,
                                   wd_pre=wd_pre)

    nc.compile()
    return nc


_NC_CACHE = {}


def _get_nc(t_tokens=T, n_experts=NE):
    key = (t_tokens, n_experts)
    if key not in _NC_CACHE:
        _NC_CACHE[key] = build_moe_nc(t_tokens, n_experts)
    return _NC_CACHE[key]


def _prepare_host(x, norm_scale, gate_w, w_up, b_up, w_down, b_down):
    """Routing + weight gather/quantization on host. Returns per-core in_maps."""
    x = np.asarray(x, dtype=np.float32)
    norm_scale = np.asarray(norm_scale, dtype=np.float32)
    gate_w = np.asarray(gate_w, dtype=np.float32)

    assert not np.any(np.asarray(b_up)), "kernel assumes b_up == 0"
    assert not np.any(np.asarray(b_down)), "kernel assumes b_down == 0"

    x00 = x.reshape(-1, H)[0].astype(np.float64)
    rstd = 1.0 / np.sqrt(np.mean(x00 * x00) + EPS)
    xn00 = x00 * rstd * norm_scale.astype(np.float64)
    logits00 = gate_w.astype(np.float64) @ xn00
    eids = np.argsort(-logits00, kind="stable")[:NE] % E

    wu = np.asarray(w_up, dtype=np.float32)[eids]     # [NE, I2, H]
    wd = np.asarray(w_down, dtype=np.float32)[eids]   # [NE, H, I]
    gw = gate_w
    if not np.all(norm_scale == 1.0):
        # fold the RMSNorm scale into every weight that contracts over H
        wu = wu * norm_scale[None, None, :]
        gw = gate_w * norm_scale[None, :]

    NPAIR = I2 // 256
    wu_s = wu * np.float32(WS)
    wg, wl = wu_s[:, :I], wu_s[:, I:]          # SwiGLU gate / linear halves
    # [e, i2h, h] -> [e, j, m, u, q, p]; device wants [e, j, p, u(pairs), q, m]
    wg_r = wg.reshape(NE, NPAIR, 128, HT // 2, 2, 128)
    wl_r = wl.reshape(NE, NPAIR, 128, HT // 2, 2, 128)
    wu8gD = np.ascontiguousarray(
        wg_r[:, :, :, :NDR_G].transpose(0, 1, 5, 3, 4, 2)
    )
    wu8gD = np.clip(wu8gD, -F8MAX, F8MAX).astype(NP_F8)
    wu8lD = np.ascontiguousarray(
        wl_r[:, :, :, :NDR_L].transpose(0, 1, 5, 3, 4, 2)
    )
    wu8lD = np.clip(wu8lD, -F8MAX, F8MAX).astype(NP_F8)
    # f32r tail of the g contraction (also scaled by WS — exact)
    wufgD = np.ascontiguousarray(
        wg[:, :, 2 * NDR_G * 128:]
        .reshape(NE, NPAIR, 128, NF32_G, 128)
        .transpose(0, 1, 4, 3, 2)
    )

    wdT = np.ascontiguousarray(wd.transpose(0, 2, 1))  # [NE, I, H]
    gwT = np.ascontiguousarray(gw.T)                   # [H, E]
    gwD = np.ascontiguousarray(
        gwT.reshape(H // 128, 128, E).transpose(1, 0, 2)
    )                                                  # [128, HT, E]

    x_flat = np.ascontiguousarray(x.reshape(T_FULL, H))
    in_maps = []
    for c in range(N_CORES):
        in_maps.append(
            {
                "x_sh": x_flat[c * T:(c + 1) * T],
                "wu8g": wu8gD,
                "wufg": wufgD,
                "wu8l": wu8lD,
                "wdT": wdT,
                "gwD": gwD,
            }
        )
    return in_maps, x.shape


def run_moe(inputs, trace=False, **run_kwargs):
    in_maps, x_shape = _prepare_host(**inputs)
    nc = _get_nc()
    br = run_bass_kernel_spmd(
        nc, in_maps, core_ids=list(range(N_CORES)), trace=trace, **run_kwargs
    )
    out = np.concatenate([r["out_sh"] for r in br.results], axis=0)
    return out.reshape(x_shape), br


def kernel(**inputs) -> np.ndarray:
    out, _ = run_moe(inputs, trace=False)
    return out
